# revision 74
# baseline (speedup 1.0000x reference)
"""Trainium2 Bass kernel for windowed multi-head attention (Swin-style block).

Reference computation (per batch window b of 128, N=196 tokens, C=768, H=12 heads):
    qkv  = x @ qkv_w.T + [q_bias, 0, v_bias]
    q,k,v = split(qkv);  attn = softmax(q*scale @ k.T + rel_pos_bias)
    out  = (attn @ v) @ proj_w.T + proj_b

Sharding: data-parallel over batch across 8 cores (16 windows/core).

Per-core kernel layout strategy (bf16 matmul datapath, fp32 PSUM accumulate):
  - x [196,768] is PE-transposed once to xT [768,196] bf16 (lhsT/rhs source).
  - Q^T,K^T [c',t] = W^T chunks (lhsT) x xT (rhs); V [t,c'] = xT (lhsT) x W^T (rhs).
  - S^T[j,i] = K^T-head (lhsT) x Q^T-head (rhs), K=64 contraction; heads pairs
    are row-tiled (partition base 0/64); their psum banks are separate.
  - E^T = exp(0.125*S^T) * exp(bias)^T  (ACT exp from PSUM -> bf16, DVE multiply
    with a host-precomputed exp(bias) table -- exp(a+b)=exp(a)exp(b)).
  - O natural [i, h*d] per (i-chunk, head-group-of-6) = E^T chunk (lhsT) x
    [V-head | ones] (rhs): psum column 64 of each head block is the softmax
    denominator, i.e. a per-partition scalar -> tiny DVE reciprocal;
    normalization is a stride-0-broadcast DVE multiply fused into the
    PSUM->SBUF eviction, so each O group frees its psum bank promptly.
  - O is PE-transposed back to O^T chunks (f32) for the projection.
  - y = O^T chunks (lhsT) x P^T (rhs) + proj_b (DVE add with broadcast bias).
Software pipelining (in-order engines -> emission order is the schedule):
  - per pair: MID = QK chunks interleaved with S/exp/mul head groups (ACT exp
    latency hides behind QK matmuls), then V; POST = fine-grained O groups
    interleaved across the two windows, O transposes (evictions alternate
    ACT/DVE), the NEXT pair's x-transposes (their ACT evictions hide under the
    projections), then both projections. x DMA is prefetched one pair ahead.
  - PSUM: one shared 4-deep [128,512]f32 ring (ps_mm) for QK/V/O/transposes/
    proj + a 2-deep 2-bank ring (ps_s) for S.
Hardware notes: matmuls at different partition bases must not share a psum
bank (device-fatal); K=64 head-pair matmuls alternate PE row groups 0/64.
"""

import sys

import numpy as np

if "/opt/trn_rl_repo" not in sys.path:
    sys.path.insert(0, "/opt/trn_rl_repo")

import concourse.bass as bass  # noqa: E402
import concourse.mybir as mybir  # noqa: E402
import concourse.tile as tile  # noqa: E402
from concourse import bacc  # noqa: E402
from concourse import bass_utils  # noqa: E402
from concourse.masks import make_identity  # noqa: E402

# Problem shapes (hardcoded; kernel.py must be self-contained).
B, N, C = 128, 196, 768
H, HD = 12, 64
WS = 14
NCORES = 8
BW = B // NCORES  # 16 windows per core
NPAIRS = BW // 2
JC = 98  # j/t chunk size (2 chunks per 196-token window)
F32 = mybir.dt.float32
BF16 = mybir.dt.bfloat16
SCALE = HD ** -0.5  # 0.125


def _relative_position_index(ws: int) -> np.ndarray:
    coords = np.stack(np.meshgrid(np.arange(ws), np.arange(ws), indexing="ij"))
    flat = coords.reshape(2, -1)
    rel = flat[:, :, None] - flat[:, None, :]
    rel = rel.transpose(1, 2, 0).copy()
    rel[..., 0] += ws - 1
    rel[..., 1] += ws - 1
    rel[..., 0] *= 2 * ws - 1
    return rel.sum(-1)  # [N, N] int


def _build_kernel_body(ctx, tc, aps, reps=1):
    nc = tc.nc
    x_d = aps["x_sh"]
    wT_d = aps["wT"]
    pT_d = aps["pT"]
    qb_d = aps["qb"]
    pb_d = aps["pb"]
    eb_d = aps["expBT"]
    y_d = aps["y_sh"]

    const = ctx.enter_context(tc.tile_pool(name="const", bufs=1))

    # ---- resident constants ----
    w_sb = const.tile([128, 6, 3 * C], BF16)  # W^T: [c%128, c//128, c']
    for wc in range(3):  # split Q/K/V so the first QK doesn't wait for all
        nc.sync.dma_start(
            out=w_sb[:, :, wc * C : (wc + 1) * C],
            in_=wT_d.rearrange("(a p) m -> p a m", p=128)[
                :, :, wc * C : (wc + 1) * C])
    pT_sb = const.tile([128, 6, C], BF16)
    nc.sync.dma_start(out=pT_sb, in_=pT_d.rearrange("(a p) m -> p a m", p=128))
    qb_sb = const.tile([128, 6], F32)
    nc.sync.dma_start(out=qb_sb, in_=qb_d.rearrange("(a p) -> p a", p=128))
    def _bcast(src, parts):
        return bass.AP(tensor=src.tensor, offset=src.offset,
                       ap=[[0, parts]] + list(src.ap))

    # v_bias is folded into pb on the host (attn rows sum to 1 after the
    # ones-column normalize, so attn@(V+vb) = attn@V + vb, and the constant
    # vb term passes through the projection as vb @ proj_w.T).
    pb_bc = const.tile([128, C], F32)
    nc.sync.dma_start(out=pb_bc, in_=_bcast(pb_d, 128))
    eb_sb = const.tile([JC, H, 2 * N], BF16)  # exp(bias)^T: [j%98, h, (j//98)*196+i]
    nc.sync.dma_start(out=eb_sb, in_=eb_d.rearrange("p (h m) -> p h m", h=H))
    ident = const.tile([128, 128], F32)
    make_identity(nc, ident)

    # ---- pools ----
    xin = ctx.enter_context(tc.tile_pool(name="xin", bufs=4))
    xt = ctx.enter_context(tc.tile_pool(name="xt", bufs=3))
    qk = ctx.enter_context(tc.tile_pool(name="qk", bufs=2))
    vpool = ctx.enter_context(tc.tile_pool(name="vpool", bufs=2))
    epool = ctx.enter_context(tc.tile_pool(name="epool", bufs=3))
    opool = ctx.enter_context(tc.tile_pool(name="opool", bufs=2))
    onp = ctx.enter_context(tc.tile_pool(name="onp", bufs=3))
    rpool = ctx.enter_context(tc.tile_pool(name="rpool", bufs=2))
    ypool = ctx.enter_context(tc.tile_pool(name="ypool", bufs=2))
    ps_mm = ctx.enter_context(tc.tile_pool(name="ps_mm", bufs=4, space="PSUM"))
    ps_s = ctx.enter_context(tc.tile_pool(name="ps_s", bufs=2, space="PSUM"))

    # ---------------- stage helpers ----------------
    def fetch_x(pi):
        """One DMA per window into [98, 2, C]."""
        tws = []
        for w in (2 * pi, 2 * pi + 1):
            tw = xin.tile([JC, 2, C], F32, tag="xw")
            nc.sync.dma_start(
                out=tw, in_=x_d[w].rearrange("(a p) m -> p a m", p=JC))
            tws.append(tw)
        return tws

    def stage_pre(xw):
        """Transpose x -> xT (f32 in, bf16 out via the ACT eviction)."""
        xT = xt.tile([128, 6, 2 * N], BF16)
        for ci in range(6):
            pt = ps_mm.tile([128, 512], F32, tag="mm")
            for wi in range(2):
                for tck in range(2):
                    nc.tensor.transpose(
                        pt[:, wi * N + tck * JC : wi * N + (tck + 1) * JC],
                        xw[wi][:, tck, ci * 128 : (ci + 1) * 128],
                        ident[0:JC, 0:JC],
                    )
            nc.scalar.copy(out=xT[:, ci, :], in_=pt[:, 0 : 2 * N])
        return xT

    def qk_chunk(qk_sb, xT, cp):
        """One Q^T/K^T output chunk (c' rows 128cp..) for the pair."""
        ps = ps_mm.tile([128, 512], F32, tag="mm")
        for ck in range(6):
            nc.tensor.matmul(
                ps[:, 0 : 2 * N],
                w_sb[:, ck, cp * 128 : (cp + 1) * 128],
                xT[:, ck, :],
                start=(ck == 0),
                stop=(ck == 5),
            )
        if cp < 6:  # Q: add q_bias (per-partition scalar) on DVE
            nc.vector.tensor_scalar_add(
                out=qk_sb[:, cp, :], in0=ps[:, 0 : 2 * N],
                scalar1=qb_sb[:, cp : cp + 1],
            )
        else:  # K: plain copy, also DVE (keeps MID-phase ACT exp-only)
            nc.vector.tensor_copy(out=qk_sb[:, cp, :], in_=ps[:, 0 : 2 * N])

    def s_group(qk_sb, e2a, wi, g):
        """S^T matmuls + exp + bias-mul for head pair g of window wi."""
        woff = wi * N
        pss = ps_s.tile([128, 2, 512], F32, tag="s")
        for jc in range(2):
            for hh in range(2):  # hh inner: alternate PE row groups
                h = 2 * g + hh
                prow = (h % 2) * 64
                nc.tensor.matmul(
                    pss[0:JC, hh, jc * N : (jc + 1) * N],
                    qk_sb[prow : prow + 64, 6 + h // 2,
                          woff + jc * JC : woff + (jc + 1) * JC],
                    qk_sb[prow : prow + 64, h // 2, woff : woff + N],
                    start=True,
                    stop=True,
                )
        nc.scalar.activation(
            out=e2a[:, g],
            in_=pss[0:JC, :, 0 : 2 * N].rearrange("p b (a n) -> p b a n", a=2),
            func=mybir.ActivationFunctionType.Exp,
            scale=SCALE,
        )
        nc.vector.tensor_mul(
            e2a[:, g],
            e2a[:, g],
            eb_sb[:, 2 * g : 2 * g + 2, :].rearrange("p b (a n) -> p b a n",
                                                     a=2),
        )

    def v_window(xT, wi):
        """V natural (+v_bias, +ones column for denominators)."""
        vt = vpool.tile([128, 2, H, HD + 1], BF16, tag="v")
        nc.gpsimd.memset(vt[0:JC, :, :, HD : HD + 1], 1.0)
        for tck in range(2):
            for c0, nn in ((0, 512), (512, 256)):
                ps = ps_mm.tile([128, 512], F32, tag="mm")
                for ck in range(6):
                    nc.tensor.matmul(
                        ps[0:JC, 0:nn],
                        xT[:, ck, wi * N + tck * JC : wi * N + (tck + 1) * JC],
                        w_sb[:, ck, 2 * C + c0 : 2 * C + c0 + nn],
                        start=(ck == 0),
                        stop=(ck == 5),
                    )
                h0 = c0 // HD
                nh = nn // HD
                veng = nc.scalar.copy if c0 == 0 else nc.vector.tensor_copy
                veng(
                    out=vt[0:JC, tck, h0 : h0 + nh, 0:HD],
                    in_=ps[0:JC, 0:nn].rearrange("p (h d) -> p h d", d=HD),
                )
        return vt

    def o_group(e2a, vt, onat, dens, rcp, ic, hb):
        """O natural for one (i-chunk, head-group): matmuls + dens/recip +
        norm-evict. PSUM comes from the shared 4-deep mm ring ([98,
        6*(HD+1)] fits one bank). Self-contained: dens/recip/norm free the
        bank promptly so groups can be woven into the MID phase."""
        pso = ps_mm.tile([128, 512], F32, tag="mm")
        for hi in range(6):
            h = 6 * hb + hi
            g, hh = divmod(h, 2)
            for jc in range(2):
                nc.tensor.matmul(
                    pso[0:JC, hi * (HD + 1) : (hi + 1) * (HD + 1)],
                    e2a[0:JC, g, hh, jc, ic * JC : (ic + 1) * JC],
                    vt[0:JC, jc, h, :],
                    start=(jc == 0),
                    stop=(jc == 1),
                )
        nc.vector.tensor_copy(
            out=dens[:, ic, hb, :],
            in_=pso[0:JC, 0 : 6 * (HD + 1)].rearrange(
                "p (h e) -> p h e", e=HD + 1)[:, :, HD])
        nc.vector.reciprocal(out=rcp[:, ic, hb, :], in_=dens[:, ic, hb, :])
        rb = rcp[:, ic, hb, :]  # [98, 6]; broadcast over d
        rb_bc = bass.AP(tensor=rb.tensor, offset=rb.offset,
                        ap=list(rb.ap) + [[0, HD]])
        nc.vector.tensor_mul(
            onat[:, ic, hb, :].rearrange("p (h d) -> p h d", d=HD),
            pso[0:JC, 0 : 6 * (HD + 1)].rearrange(
                "p (h e) -> p h e", e=HD + 1)[:, :, 0:HD],
            rb_bc,
        )

    def o_trans(oc, onat, cis):
        """Transpose O -> O^T chunks (f32 via ps_mm) for the given ci range
        (ci 0-2 need only head-group 0, 3-5 only group 1, so halves can be
        woven between O groups). Evictions alternate ACT/DVE so the proj
        consumer isn't paced by one engine's copies."""
        for ci in cis:
            hb, coff = divmod(ci * 128, 6 * HD)
            pt = ps_mm.tile([128, 512], F32, tag="mm")
            for ic in range(2):
                nc.tensor.transpose(
                    pt[:, ic * JC : (ic + 1) * JC],
                    onat[:, ic, hb, coff : coff + 128],
                    ident[0:JC, 0:JC],
                )
            if ci % 2 == 0:
                nc.scalar.copy(out=oc[:, ci, :], in_=pt[:, 0:N])
            else:
                nc.vector.tensor_copy(out=oc[:, ci, :], in_=pt[:, 0:N])

    def proj_window(oc, w):
        """Projection + bias + y DMA for one window."""
        y2 = ypool.tile([JC, 2, C], F32, tag="y")
        for tck in range(2):
            for c0, nn in ((0, 512), (512, 256)):
                ps = ps_mm.tile([128, 512], F32, tag="mm")
                for ck in range(6):
                    nc.tensor.matmul(
                        ps[0:JC, 0:nn],
                        oc[:, ck, tck * JC : (tck + 1) * JC],
                        pT_sb[:, ck, c0 : c0 + nn],
                        start=(ck == 0),
                        stop=(ck == 5),
                    )
                nc.vector.tensor_add(
                    out=y2[0:JC, tck, c0 : c0 + nn],
                    in0=ps[0:JC, 0:nn],
                    in1=pb_bc[0:JC, c0 : c0 + nn],
                )
        nc.sync.dma_start(
            out=y_d[w].rearrange("(a p) m -> p a m", p=JC), in_=y2)

    # ---------------- software-pipelined pair loop ----------------
    # Emission order per pair: MID (QK interleaved with S/exp so the ACT
    # exp latency hides behind QK matmuls; then V), PRE of the NEXT pair
    # (its ACT evictions hide under POST's PE work), POST (O chunks
    # interleaved across the two windows, then transpose+proj+DMA).
    seq = [pi for _ in range(reps) for pi in range(NPAIRS)]
    xw_cur = fetch_x(seq[0])
    xT_cur = stage_pre(xw_cur)
    for k, pi in enumerate(seq):
        wins = (2 * pi, 2 * pi + 1)
        if k + 1 < len(seq):
            xw_next = fetch_x(seq[k + 1])
        # MID: QK chunks interleaved with S/exp/mul blocks, then V.
        qk_sb = qk.tile([128, 12, 2 * N], BF16)
        e2as = [epool.tile([JC, 6, 2, 2, N], BF16, tag="e", name=f"e2a{wi}")
                for wi in range(2)]
        onats, denss, rcps, v_ts = [], [], [], []
        for wi in range(2):
            onats.append(onp.tile([JC, 2, 2, 6 * HD], F32, tag="on",
                                  name=f"onat{wi}"))
            denss.append(rpool.tile([JC, 2, 2, 6], F32, tag="den",
                                    name=f"dens{wi}"))
            rcps.append(rpool.tile([JC, 2, 2, 6], F32, tag="rcp",
                                   name=f"rcp{wi}"))
        for g in range(6):
            qk_chunk(qk_sb, xT_cur, g)
            qk_chunk(qk_sb, xT_cur, 6 + g)
            for wi in range(2):
                s_group(qk_sb, e2as[wi], wi, g)
        for wi in range(2):
            v_ts.append(v_window(xT_cur, wi))
        # POST: fine-grained O groups interleaved across windows (each frees
        # its psum bank via its own dens/recip/norm), with O-transpose
        # halves woven in so every DVE norm tail hides under PE work and
        # the oc copies land well before the projections consume them.
        oc0 = opool.tile([128, 6, N], BF16, tag="oc", name="oc0")
        oc1 = opool.tile([128, 6, N], BF16, tag="oc", name="oc1")

        def og(wi, ic, hb):
            o_group(e2as[wi], v_ts[wi], onats[wi], denss[wi], rcps[wi],
                    ic, hb)

        for hb in range(2):
            for wi in range(2):
                for ic in range(2):
                    og(wi, ic, hb)
        o_trans(oc0, onats[0], range(6))
        o_trans(oc1, onats[1], range(6))
        if k + 1 < len(seq):
            xT_next = stage_pre(xw_next)
        proj_window(oc0, wins[0])
        proj_window(oc1, wins[1])
        if k + 1 < len(seq):
            xw_cur, xT_cur = xw_next, xT_next


def build_program(reps=1):
    """Build + compile the per-core Bass program. Returns the Bacc instance."""
    nc = bacc.Bacc(
        "TRN2",
        target_bir_lowering=False,
        debug=False,
        enable_asserts=False,
        num_devices=NCORES,
    )
    aps = {
        "x_sh": nc.dram_tensor("x_sh", [BW, N, C], F32, kind="ExternalInput").ap(),
        "wT": nc.dram_tensor("wT", [C, 3 * C], BF16, kind="ExternalInput").ap(),
        "pT": nc.dram_tensor("pT", [C, C], BF16, kind="ExternalInput").ap(),
        "qb": nc.dram_tensor("qb", [C], F32, kind="ExternalInput").ap(),
        "pb": nc.dram_tensor("pb", [C], F32, kind="ExternalInput").ap(),
        "expBT": nc.dram_tensor(
            "expBT", [JC, H * 2 * N], BF16, kind="ExternalInput").ap(),
        "y_sh": nc.dram_tensor("y_sh", [BW, N, C], F32, kind="ExternalOutput").ap(),
    }

    from contextlib import ExitStack

    with tile.TileContext(nc) as tc:
        with ExitStack() as ctx:
            _build_kernel_body(ctx, tc, aps, reps=reps)
    nc.compile()
    return nc


_CACHED = {}


def _get_program(reps=1):
    key = f"nc{reps}"
    if key not in _CACHED:
        _CACHED[key] = build_program(reps=reps)
    return _CACHED[key]


def host_prep(qkv_w, q_bias, v_bias, rpb_table, proj_w, proj_b):
    """Host-side constant layout prep (shared across cores)."""
    idx = _relative_position_index(WS)  # [N, N] ints
    bias = rpb_table[idx.reshape(-1)].reshape(N, N, H)  # [i, j, h]
    expB = np.exp(bias.astype(np.float32))
    # expBT[r, h, jc*N + i] = expB[i, jc*JC + r, h]
    e = expB.transpose(2, 1, 0).reshape(H, 2, JC, N)  # [h, jc, r, i]
    expBT = np.ascontiguousarray(e.transpose(2, 0, 1, 3)).reshape(JC, H * 2 * N)
    import ml_dtypes

    bf16 = ml_dtypes.bfloat16
    fp8 = ml_dtypes.float8_e4m3fn
    return {
        "wT": np.ascontiguousarray(qkv_w.T).astype(bf16),
        "pT": np.ascontiguousarray(proj_w.T).astype(bf16),
        "qb": np.ascontiguousarray(q_bias, np.float32),
        "pb": np.ascontiguousarray(
            np.asarray(proj_b, np.float64)
            + np.asarray(v_bias, np.float64) @ np.asarray(proj_w, np.float64).T
        ).astype(np.float32),
        "expBT": expBT.astype(bf16),
    }


def make_in_maps(x, qkv_w, q_bias, v_bias, rpb_table, proj_w, proj_b):
    shared = host_prep(qkv_w, q_bias, v_bias, rpb_table, proj_w, proj_b)
    x_f = np.asarray(x, np.float32)
    in_maps = []
    for ci in range(NCORES):
        m = dict(shared)
        m["x_sh"] = np.ascontiguousarray(x_f[ci * BW : (ci + 1) * BW])
        in_maps.append(m)
    return in_maps


def kernel(x, qkv_w, q_bias, v_bias, rpb_table, proj_w, proj_b, _trace=False):
    """Full-input entry point: shards over 8 NeuronCores, returns full output."""
    nc = _get_program()
    in_maps = make_in_maps(x, qkv_w, q_bias, v_bias, rpb_table, proj_w, proj_b)
    res = bass_utils.run_bass_kernel_spmd(
        nc, in_maps, core_ids=list(range(NCORES)), trace=_trace)
    out = np.concatenate([res.results[i]["y_sh"] for i in range(NCORES)], axis=0)
    if _trace:
        return out, res
    return out



# revision 78
# speedup vs baseline: 1.0034x; 1.0034x over previous
"""Trainium2 Bass kernel for windowed multi-head attention (Swin-style block).

Reference computation (per batch window b of 128, N=196 tokens, C=768, H=12 heads):
    qkv  = x @ qkv_w.T + [q_bias, 0, v_bias]
    q,k,v = split(qkv);  attn = softmax(q*scale @ k.T + rel_pos_bias)
    out  = (attn @ v) @ proj_w.T + proj_b

Sharding: data-parallel over batch across 8 cores (16 windows/core).

Per-core kernel layout strategy (bf16 matmul datapath, fp32 PSUM accumulate):
  - x [196,768] is PE-transposed once to xT [768,196] bf16 (lhsT/rhs source).
  - Q^T,K^T [c',t] = W^T chunks (lhsT) x xT (rhs); V [t,c'] = xT (lhsT) x W^T (rhs).
  - S^T[j,i] = K^T-head (lhsT) x Q^T-head (rhs), K=64 contraction; heads pairs
    are row-tiled (partition base 0/64); their psum banks are separate.
  - E^T = exp(0.125*S^T) * exp(bias)^T  (ACT exp from PSUM -> bf16; the
    multiply by the host-precomputed exp(bias) table is split DVE/GPSIMD
    per head -- exp(a+b)=exp(a)exp(b)).
  - O natural [i, h*d] per (i-chunk, head-group-of-6) = E^T chunk (lhsT) x
    [V-head | ones] (rhs): psum column 64 of each head block is the softmax
    denominator, i.e. a per-partition scalar -> tiny DVE reciprocal;
    normalization is a stride-0-broadcast DVE multiply fused into the
    PSUM->SBUF eviction, so each O group frees its psum bank promptly.
  - O is PE-transposed back to O^T chunks (f32) for the projection.
  - y = O^T chunks (lhsT) x P^T (rhs) + proj_b (DVE add with broadcast bias).
Software pipelining (in-order engines -> emission order is the schedule):
  - per pair: MID = QK chunks interleaved with S/exp/mul head groups (ACT exp
    latency hides behind QK matmuls), then V; POST = fine-grained O groups
    interleaved across the two windows, O transposes (evictions alternate
    ACT/DVE), the NEXT pair's x-transposes (their ACT evictions hide under the
    projections), then both projections. x DMA is prefetched one pair ahead.
  - PSUM: one shared 4-deep [128,512]f32 ring (ps_mm) for QK/V/O/transposes/
    proj + a 2-deep 2-bank ring (ps_s) for S.
Hardware notes: matmuls at different partition bases must not share a psum
bank (device-fatal); K=64 head-pair matmuls alternate PE row groups 0/64.
"""

import sys

import numpy as np

if "/opt/trn_rl_repo" not in sys.path:
    sys.path.insert(0, "/opt/trn_rl_repo")

import concourse.bass as bass  # noqa: E402
import concourse.mybir as mybir  # noqa: E402
import concourse.tile as tile  # noqa: E402
from concourse import bacc  # noqa: E402
from concourse import bass_utils  # noqa: E402
from concourse.masks import make_identity  # noqa: E402

# Problem shapes (hardcoded; kernel.py must be self-contained).
B, N, C = 128, 196, 768
H, HD = 12, 64
WS = 14
NCORES = 8
BW = B // NCORES  # 16 windows per core
NPAIRS = BW // 2
JC = 98  # j/t chunk size (2 chunks per 196-token window)
F32 = mybir.dt.float32
BF16 = mybir.dt.bfloat16
SCALE = HD ** -0.5  # 0.125


def _relative_position_index(ws: int) -> np.ndarray:
    coords = np.stack(np.meshgrid(np.arange(ws), np.arange(ws), indexing="ij"))
    flat = coords.reshape(2, -1)
    rel = flat[:, :, None] - flat[:, None, :]
    rel = rel.transpose(1, 2, 0).copy()
    rel[..., 0] += ws - 1
    rel[..., 1] += ws - 1
    rel[..., 0] *= 2 * ws - 1
    return rel.sum(-1)  # [N, N] int


def _build_kernel_body(ctx, tc, aps, reps=1):
    nc = tc.nc
    x_d = aps["x_sh"]
    wT_d = aps["wT"]
    pT_d = aps["pT"]
    qb_d = aps["qb"]
    pb_d = aps["pb"]
    eb_d = aps["expBT"]
    y_d = aps["y_sh"]

    const = ctx.enter_context(tc.tile_pool(name="const", bufs=1))

    # ---- resident constants ----
    w_sb = const.tile([128, 6, 3 * C], BF16)  # W^T: [c%128, c//128, c']
    for wc in range(3):  # split Q/K/V so the first QK doesn't wait for all
        nc.sync.dma_start(
            out=w_sb[:, :, wc * C : (wc + 1) * C],
            in_=wT_d.rearrange("(a p) m -> p a m", p=128)[
                :, :, wc * C : (wc + 1) * C])
    pT_sb = const.tile([128, 6, C], BF16)
    nc.sync.dma_start(out=pT_sb, in_=pT_d.rearrange("(a p) m -> p a m", p=128))
    qb_sb = const.tile([128, 6], F32)
    nc.sync.dma_start(out=qb_sb, in_=qb_d.rearrange("(a p) -> p a", p=128))
    def _bcast(src, parts):
        return bass.AP(tensor=src.tensor, offset=src.offset,
                       ap=[[0, parts]] + list(src.ap))

    # v_bias is folded into pb on the host (attn rows sum to 1 after the
    # ones-column normalize, so attn@(V+vb) = attn@V + vb, and the constant
    # vb term passes through the projection as vb @ proj_w.T).
    pb_bc = const.tile([128, C], F32)
    nc.sync.dma_start(out=pb_bc, in_=_bcast(pb_d, 128))
    eb_sb = const.tile([JC, H, 2 * N], BF16)  # exp(bias)^T: [j%98, h, (j//98)*196+i]
    nc.sync.dma_start(out=eb_sb, in_=eb_d.rearrange("p (h m) -> p h m", h=H))
    ident = const.tile([128, 128], F32)
    make_identity(nc, ident)

    # ---- pools ----
    xin = ctx.enter_context(tc.tile_pool(name="xin", bufs=4))
    xt = ctx.enter_context(tc.tile_pool(name="xt", bufs=3))
    qk = ctx.enter_context(tc.tile_pool(name="qk", bufs=2))
    vpool = ctx.enter_context(tc.tile_pool(name="vpool", bufs=2))
    epool = ctx.enter_context(tc.tile_pool(name="epool", bufs=3))
    opool = ctx.enter_context(tc.tile_pool(name="opool", bufs=2))
    onp = ctx.enter_context(tc.tile_pool(name="onp", bufs=3))
    rpool = ctx.enter_context(tc.tile_pool(name="rpool", bufs=2))
    ypool = ctx.enter_context(tc.tile_pool(name="ypool", bufs=2))
    ps_mm = ctx.enter_context(tc.tile_pool(name="ps_mm", bufs=4, space="PSUM"))
    ps_s = ctx.enter_context(tc.tile_pool(name="ps_s", bufs=2, space="PSUM"))

    # ---------------- stage helpers ----------------
    def fetch_x(pi):
        """One DMA per window into [98, 2, C]."""
        tws = []
        for w in (2 * pi, 2 * pi + 1):
            tw = xin.tile([JC, 2, C], F32, tag="xw")
            nc.sync.dma_start(
                out=tw, in_=x_d[w].rearrange("(a p) m -> p a m", p=JC))
            tws.append(tw)
        return tws

    def stage_pre(xw):
        """Transpose x -> xT (f32 in, bf16 out via the ACT eviction)."""
        xT = xt.tile([128, 6, 2 * N], BF16)
        for ci in range(6):
            pt = ps_mm.tile([128, 512], F32, tag="mm")
            for wi in range(2):
                for tck in range(2):
                    nc.tensor.transpose(
                        pt[:, wi * N + tck * JC : wi * N + (tck + 1) * JC],
                        xw[wi][:, tck, ci * 128 : (ci + 1) * 128],
                        ident[0:JC, 0:JC],
                    )
            nc.scalar.copy(out=xT[:, ci, :], in_=pt[:, 0 : 2 * N])
        return xT

    def qk_chunk(qk_sb, xT, cp):
        """One Q^T/K^T output chunk (c' rows 128cp..) for the pair."""
        ps = ps_mm.tile([128, 512], F32, tag="mm")
        for ck in range(6):
            nc.tensor.matmul(
                ps[:, 0 : 2 * N],
                w_sb[:, ck, cp * 128 : (cp + 1) * 128],
                xT[:, ck, :],
                start=(ck == 0),
                stop=(ck == 5),
            )
        if cp < 6:  # Q: add q_bias (per-partition scalar) on DVE
            nc.vector.tensor_scalar_add(
                out=qk_sb[:, cp, :], in0=ps[:, 0 : 2 * N],
                scalar1=qb_sb[:, cp : cp + 1],
            )
        else:  # K: plain copy, also DVE (keeps MID-phase ACT exp-only)
            nc.vector.tensor_copy(out=qk_sb[:, cp, :], in_=ps[:, 0 : 2 * N])

    def s_group(qk_sb, e2a, wi, g):
        """S^T matmuls + exp + bias-mul for head pair g of window wi."""
        woff = wi * N
        pss = ps_s.tile([128, 2, 512], F32, tag="s")
        for jc in range(2):
            for hh in range(2):  # hh inner: alternate PE row groups
                h = 2 * g + hh
                prow = (h % 2) * 64
                nc.tensor.matmul(
                    pss[0:JC, hh, jc * N : (jc + 1) * N],
                    qk_sb[prow : prow + 64, 6 + h // 2,
                          woff + jc * JC : woff + (jc + 1) * JC],
                    qk_sb[prow : prow + 64, h // 2, woff : woff + N],
                    start=True,
                    stop=True,
                )
        nc.scalar.activation(
            out=e2a[:, g],
            in_=pss[0:JC, :, 0 : 2 * N].rearrange("p b (a n) -> p b a n", a=2),
            func=mybir.ActivationFunctionType.Exp,
            scale=SCALE,
        )
        # bias-mul split DVE/Pool: halves the DVE share (Pool is idle)
        nc.vector.tensor_mul(
            e2a[:, g, 0:1],
            e2a[:, g, 0:1],
            eb_sb[:, 2 * g : 2 * g + 1, :].rearrange("p b (a n) -> p b a n",
                                                     a=2),
        )
        nc.gpsimd.tensor_mul(
            e2a[:, g, 1:2],
            e2a[:, g, 1:2],
            eb_sb[:, 2 * g + 1 : 2 * g + 2, :].rearrange(
                "p b (a n) -> p b a n", a=2),
        )

    def v_window(xT, wi):
        """V natural (+v_bias, +ones column for denominators)."""
        vt = vpool.tile([128, 2, H, HD + 1], BF16, tag="v")
        nc.gpsimd.memset(vt[0:JC, :, :, HD : HD + 1], 1.0)
        for tck in range(2):
            for c0, nn in ((0, 512), (512, 256)):
                ps = ps_mm.tile([128, 512], F32, tag="mm")
                for ck in range(6):
                    nc.tensor.matmul(
                        ps[0:JC, 0:nn],
                        xT[:, ck, wi * N + tck * JC : wi * N + (tck + 1) * JC],
                        w_sb[:, ck, 2 * C + c0 : 2 * C + c0 + nn],
                        start=(ck == 0),
                        stop=(ck == 5),
                    )
                h0 = c0 // HD
                nh = nn // HD
                veng = nc.scalar.copy if c0 == 0 else nc.vector.tensor_copy
                veng(
                    out=vt[0:JC, tck, h0 : h0 + nh, 0:HD],
                    in_=ps[0:JC, 0:nn].rearrange("p (h d) -> p h d", d=HD),
                )
        return vt

    def o_group(e2a, vt, onat, dens, rcp, ic, hb):
        """O natural for one (i-chunk, head-group): matmuls + dens/recip +
        norm-evict. PSUM comes from the shared 4-deep mm ring ([98,
        6*(HD+1)] fits one bank). Self-contained: dens/recip/norm free the
        bank promptly so groups can be woven into the MID phase."""
        pso = ps_mm.tile([128, 512], F32, tag="mm")
        for hi in range(6):
            h = 6 * hb + hi
            g, hh = divmod(h, 2)
            for jc in range(2):
                nc.tensor.matmul(
                    pso[0:JC, hi * (HD + 1) : (hi + 1) * (HD + 1)],
                    e2a[0:JC, g, hh, jc, ic * JC : (ic + 1) * JC],
                    vt[0:JC, jc, h, :],
                    start=(jc == 0),
                    stop=(jc == 1),
                )
        nc.vector.tensor_copy(
            out=dens[:, ic, hb, :],
            in_=pso[0:JC, 0 : 6 * (HD + 1)].rearrange(
                "p (h e) -> p h e", e=HD + 1)[:, :, HD])
        nc.vector.reciprocal(out=rcp[:, ic, hb, :], in_=dens[:, ic, hb, :])
        rb = rcp[:, ic, hb, :]  # [98, 6]; broadcast over d
        rb_bc = bass.AP(tensor=rb.tensor, offset=rb.offset,
                        ap=list(rb.ap) + [[0, HD]])
        nc.vector.tensor_mul(
            onat[:, ic, hb, :].rearrange("p (h d) -> p h d", d=HD),
            pso[0:JC, 0 : 6 * (HD + 1)].rearrange(
                "p (h e) -> p h e", e=HD + 1)[:, :, 0:HD],
            rb_bc,
        )

    def o_trans(oc, onat, cis):
        """Transpose O -> O^T chunks (f32 via ps_mm) for the given ci range
        (ci 0-2 need only head-group 0, 3-5 only group 1, so halves can be
        woven between O groups). Evictions alternate ACT/DVE so the proj
        consumer isn't paced by one engine's copies."""
        for ci in cis:
            hb, coff = divmod(ci * 128, 6 * HD)
            pt = ps_mm.tile([128, 512], F32, tag="mm")
            for ic in range(2):
                nc.tensor.transpose(
                    pt[:, ic * JC : (ic + 1) * JC],
                    onat[:, ic, hb, coff : coff + 128],
                    ident[0:JC, 0:JC],
                )
            if ci % 2 == 0:
                nc.scalar.copy(out=oc[:, ci, :], in_=pt[:, 0:N])
            else:
                nc.vector.tensor_copy(out=oc[:, ci, :], in_=pt[:, 0:N])

    def proj_window(oc, w):
        """Projection + bias + y DMA for one window."""
        y2 = ypool.tile([JC, 2, C], F32, tag="y")
        for tck in range(2):
            for c0, nn in ((0, 512), (512, 256)):
                ps = ps_mm.tile([128, 512], F32, tag="mm")
                for ck in range(6):
                    nc.tensor.matmul(
                        ps[0:JC, 0:nn],
                        oc[:, ck, tck * JC : (tck + 1) * JC],
                        pT_sb[:, ck, c0 : c0 + nn],
                        start=(ck == 0),
                        stop=(ck == 5),
                    )
                nc.vector.tensor_add(
                    out=y2[0:JC, tck, c0 : c0 + nn],
                    in0=ps[0:JC, 0:nn],
                    in1=pb_bc[0:JC, c0 : c0 + nn],
                )
        nc.sync.dma_start(
            out=y_d[w].rearrange("(a p) m -> p a m", p=JC), in_=y2)

    # ---------------- software-pipelined pair loop ----------------
    # Emission order per pair: MID (QK interleaved with S/exp so the ACT
    # exp latency hides behind QK matmuls; then V), PRE of the NEXT pair
    # (its ACT evictions hide under POST's PE work), POST (O chunks
    # interleaved across the two windows, then transpose+proj+DMA).
    seq = [pi for _ in range(reps) for pi in range(NPAIRS)]
    xw_cur = fetch_x(seq[0])
    xT_cur = stage_pre(xw_cur)
    for k, pi in enumerate(seq):
        wins = (2 * pi, 2 * pi + 1)
        if k + 1 < len(seq):
            xw_next = fetch_x(seq[k + 1])
        # MID: QK chunks interleaved with S/exp/mul blocks, then V.
        qk_sb = qk.tile([128, 12, 2 * N], BF16)
        e2as = [epool.tile([JC, 6, 2, 2, N], BF16, tag="e", name=f"e2a{wi}")
                for wi in range(2)]
        onats, denss, rcps, v_ts = [], [], [], []
        for wi in range(2):
            onats.append(onp.tile([JC, 2, 2, 6 * HD], F32, tag="on",
                                  name=f"onat{wi}"))
            denss.append(rpool.tile([JC, 2, 2, 6], F32, tag="den",
                                    name=f"dens{wi}"))
            rcps.append(rpool.tile([JC, 2, 2, 6], F32, tag="rcp",
                                   name=f"rcp{wi}"))
        for g in range(6):
            qk_chunk(qk_sb, xT_cur, g)
            qk_chunk(qk_sb, xT_cur, 6 + g)
            for wi in range(2):
                s_group(qk_sb, e2as[wi], wi, g)
        for wi in range(2):
            v_ts.append(v_window(xT_cur, wi))
        # POST: fine-grained O groups interleaved across windows (each frees
        # its psum bank via its own dens/recip/norm), with O-transpose
        # halves woven in so every DVE norm tail hides under PE work and
        # the oc copies land well before the projections consume them.
        oc0 = opool.tile([128, 6, N], BF16, tag="oc", name="oc0")
        oc1 = opool.tile([128, 6, N], BF16, tag="oc", name="oc1")

        def og(wi, ic, hb):
            o_group(e2as[wi], v_ts[wi], onats[wi], denss[wi], rcps[wi],
                    ic, hb)

        for hb in range(2):
            for wi in range(2):
                for ic in range(2):
                    og(wi, ic, hb)
        o_trans(oc0, onats[0], range(6))
        o_trans(oc1, onats[1], range(6))
        if k + 1 < len(seq):
            xT_next = stage_pre(xw_next)
        proj_window(oc0, wins[0])
        proj_window(oc1, wins[1])
        if k + 1 < len(seq):
            xw_cur, xT_cur = xw_next, xT_next


def build_program(reps=1):
    """Build + compile the per-core Bass program. Returns the Bacc instance."""
    nc = bacc.Bacc(
        "TRN2",
        target_bir_lowering=False,
        debug=False,
        enable_asserts=False,
        num_devices=NCORES,
    )
    aps = {
        "x_sh": nc.dram_tensor("x_sh", [BW, N, C], F32, kind="ExternalInput").ap(),
        "wT": nc.dram_tensor("wT", [C, 3 * C], BF16, kind="ExternalInput").ap(),
        "pT": nc.dram_tensor("pT", [C, C], BF16, kind="ExternalInput").ap(),
        "qb": nc.dram_tensor("qb", [C], F32, kind="ExternalInput").ap(),
        "pb": nc.dram_tensor("pb", [C], F32, kind="ExternalInput").ap(),
        "expBT": nc.dram_tensor(
            "expBT", [JC, H * 2 * N], BF16, kind="ExternalInput").ap(),
        "y_sh": nc.dram_tensor("y_sh", [BW, N, C], F32, kind="ExternalOutput").ap(),
    }

    from contextlib import ExitStack

    with tile.TileContext(nc) as tc:
        with ExitStack() as ctx:
            _build_kernel_body(ctx, tc, aps, reps=reps)
    nc.compile()
    return nc


_CACHED = {}


def _get_program(reps=1):
    key = f"nc{reps}"
    if key not in _CACHED:
        _CACHED[key] = build_program(reps=reps)
    return _CACHED[key]


def host_prep(qkv_w, q_bias, v_bias, rpb_table, proj_w, proj_b):
    """Host-side constant layout prep (shared across cores)."""
    idx = _relative_position_index(WS)  # [N, N] ints
    bias = rpb_table[idx.reshape(-1)].reshape(N, N, H)  # [i, j, h]
    expB = np.exp(bias.astype(np.float32))
    # expBT[r, h, jc*N + i] = expB[i, jc*JC + r, h]
    e = expB.transpose(2, 1, 0).reshape(H, 2, JC, N)  # [h, jc, r, i]
    expBT = np.ascontiguousarray(e.transpose(2, 0, 1, 3)).reshape(JC, H * 2 * N)
    import ml_dtypes

    bf16 = ml_dtypes.bfloat16
    fp8 = ml_dtypes.float8_e4m3fn
    return {
        "wT": np.ascontiguousarray(qkv_w.T).astype(bf16),
        "pT": np.ascontiguousarray(proj_w.T).astype(bf16),
        "qb": np.ascontiguousarray(q_bias, np.float32),
        "pb": np.ascontiguousarray(
            np.asarray(proj_b, np.float64)
            + np.asarray(v_bias, np.float64) @ np.asarray(proj_w, np.float64).T
        ).astype(np.float32),
        "expBT": expBT.astype(bf16),
    }


def make_in_maps(x, qkv_w, q_bias, v_bias, rpb_table, proj_w, proj_b):
    shared = host_prep(qkv_w, q_bias, v_bias, rpb_table, proj_w, proj_b)
    x_f = np.asarray(x, np.float32)
    in_maps = []
    for ci in range(NCORES):
        m = dict(shared)
        m["x_sh"] = np.ascontiguousarray(x_f[ci * BW : (ci + 1) * BW])
        in_maps.append(m)
    return in_maps


def kernel(x, qkv_w, q_bias, v_bias, rpb_table, proj_w, proj_b, _trace=False):
    """Full-input entry point: shards over 8 NeuronCores, returns full output."""
    nc = _get_program()
    in_maps = make_in_maps(x, qkv_w, q_bias, v_bias, rpb_table, proj_w, proj_b)
    res = bass_utils.run_bass_kernel_spmd(
        nc, in_maps, core_ids=list(range(NCORES)), trace=_trace)
    out = np.concatenate([res.results[i]["y_sh"] for i in range(NCORES)], axis=0)
    if _trace:
        return out, res
    return out

